# revision 17
# baseline (speedup 1.0000x reference)
"""Trainium2 Bass kernel for 4-bit-quantized Linear: y = x @ dequant(Wq4).T + bias.

Sharding: tensor-parallel over out_features (11008 rows -> 8 cores x 1376,
exact split), x replicated (fed pre-transposed fp16), outputs concatenated
on host.

Per-core device kernel (v4):
  - Host pre-unpacks the int4 nibbles to a transposed uint8 layout
    wqT[k, o] (pure layout transform; values stay 4-bit codes), plus a
    norm replication s[k, o] = norm[o, k//16] (fp16).
  - Device dequant is k-major, PE-free: ACT z = (2q-15)/15, DVE W = z*s,
    written straight into the fp16 weight slabs the matmul streams from.
  - Matmul phases: (1) first N1 token-chunks on o-chunk 2 only (its W is
    smallest and dequantizes first), (2) remaining token-chunks over all
    3 o-chunks with a single x read, (3) first N1 token-chunks on
    o-chunks 0,1 (x re-read for those only).
  - fp16 matmul (PSUM fp32 accumulation over K=4096) + bias add.
"""
import numpy as np

import concourse.bass as bass
import concourse.bacc as bacc
import concourse.mybir as mybir
import concourse.tile as tile
from concourse.bass_utils import run_bass_kernel_spmd

F16, F32, U8 = mybir.dt.float16, mybir.dt.float32, mybir.dt.uint8

# Problem constants (hardcoded per contract)
TOKENS, IN, OUT = 4096, 4096, 11008
GROUP, BLOCKS, HALF = 16, 256, 8
N_CORES = 8
O_C = OUT // N_CORES            # 1376 per-core out rows, exact
KT = IN // 128                  # 32 k-slabs
TC = 256                        # tokens per x-DMA super-chunk
O_CHUNKS = [(0, 512), (512, 512), (1024, 352)]   # (offset, width)
N1 = 3                          # head token-chunks processed per-phase
KB = 4                          # k-slabs per dequant batch


def build_bass(tokens=TOKENS, in_=IN, o_c=O_C, tc_sz=TC, o_chunks=None):
    kt = in_ // 128
    if o_chunks is None:
        o_chunks = O_CHUNKS
    n_tc = tokens // tc_sz
    tl_per_tc = tc_sz // 128
    maxw = max(w for _, w in o_chunks)

    nc = bacc.Bacc("TRN2", target_bir_lowering=False, debug=False)

    xt_d = nc.dram_tensor("xt", [n_tc, 128, kt * tc_sz], F16, kind="ExternalInput")
    wq_ds = [nc.dram_tensor(f"wq{i}", [128, kt, w], U8, kind="ExternalInput")
             for i, (_, w) in enumerate(o_chunks)]
    sc_ds = [nc.dram_tensor(f"sc{i}", [128, kt, w], F16, kind="ExternalInput")
             for i, (_, w) in enumerate(o_chunks)]
    br_d = nc.dram_tensor("bias_rep", [128, o_c], F32, kind="ExternalInput")
    y_d = nc.dram_tensor("y", [tokens, o_c], F32, kind="ExternalOutput")

    with tile.TileContext(nc) as tc:
        with (
            tc.tile_pool(name="const", bufs=1) as cst,
            tc.tile_pool(name="wp", bufs=3) as wp,
            tc.tile_pool(name="q8", bufs=8) as q8p,
            tc.tile_pool(name="sc", bufs=6) as scp,
            tc.tile_pool(name="xp", bufs=3) as xp,
            tc.tile_pool(name="yp", bufs=3) as yp,
            tc.tile_pool(name="psm", bufs=6, space=bass.MemorySpace.PSUM) as psm,
        ):
            wts = {}
            par = [0]

            def dq_batch(ci, b, kb):
                """Dequantize kb k-slabs of chunk ci into wts[ci] in place."""
                o_off, o_w = o_chunks[ci]
                k0 = b * kb
                p = par[0]; par[0] += 1
                # spread transfers across DMA rings; head chunk (c2) stays
                # off the gpsimd ring, which carries bias/y at the start.
                if ci == 2:
                    qeng = nc.sync if p % 2 == 0 else nc.scalar
                    seng = nc.scalar if p % 2 == 0 else nc.sync
                else:
                    rot = (nc.sync, nc.scalar, nc.gpsimd)
                    qeng = rot[p % 3]
                    seng = rot[(p + 1) % 3]
                q = q8p.tile([128, 4, maxw], U8, tag="q")
                qeng.dma_start(q[:, :kb, :o_w], wq_ds[ci][:, k0:k0 + kb, :])
                s = scp.tile([128, 4, maxw], F16, tag="s")
                seng.dma_start(s[:, :kb, :o_w], sc_ds[ci][:, k0:k0 + kb, :])
                wsl = wts[ci][:, k0:k0 + kb, :o_w]
                # z = (2q - 15)/15 = q*(2/15) - 1, written straight into W.
                # Head chunk (c2) splits compute across vector+gpsimd for
                # latency; c0/c1 stay on vector so gpsimd's DMA-issue stream
                # (their third ring) never blocks behind compute.
                zeng = (nc.vector if (ci != 2 or p % 2 == 0) else nc.gpsimd)
                zeng.tensor_scalar(
                    wsl, q[:, :kb, :o_w], 2.0 / 15.0, -1.0,
                    mybir.AluOpType.mult, mybir.AluOpType.add)
                zeng.tensor_tensor(
                    wsl, wsl, s[:, :kb, :o_w], mybir.AluOpType.mult)

            def mm_group(xtt, tci, tl, ci, y_sb=None, y_off=0):
                """One psum accumulation group + bias add (+ y dma if own)."""
                o_off, o_w = o_chunks[ci]
                ps = psm.tile([128, maxw], F32, tag="ps")
                for k in range(kt):
                    nc.tensor.matmul(
                        ps[:, :o_w],
                        xtt[:, k, tl * 128:(tl + 1) * 128],
                        wts[ci][:, k, :o_w],
                        start=(k == 0), stop=(k == kt - 1))
                own = y_sb is None
                if own:
                    y_sb = yp.tile([128, maxw], F32, tag="y", name="ys")
                    y_off = o_off
                nc.vector.tensor_tensor(
                    y_sb[:, o_off - y_off:o_off - y_off + o_w],
                    ps[:, :o_w],
                    bias_sb[:, o_off:o_off + o_w], mybir.AluOpType.add)
                if own:
                    yeng = nc.sync if (tci + tl) % 2 == 0 else nc.scalar
                    yeng.dma_start(
                        y_d[tci * tc_sz + tl * 128:
                            tci * tc_sz + (tl + 1) * 128,
                            o_off:o_off + o_w],
                        y_sb[:, :o_w])

            def x_dma(xtt, tci, quarters=False):
                # split transfers across the two HWDGE rings
                src = xt_d[tci].rearrange("p (s t) -> p s t", s=kt)
                if quarters:
                    qn = kt // 4
                    for j in range(4):
                        eng = nc.sync if j % 2 == 0 else nc.scalar
                        eng.dma_start(xtt[:, j * qn:(j + 1) * qn],
                                      src[:, j * qn:(j + 1) * qn])
                else:
                    nc.sync.dma_start(xtt[:, :kt // 2], src[:, :kt // 2])
                    nc.scalar.dma_start(xtt[:, kt // 2:], src[:, kt // 2:])

            # first x tile + bias lead the queues
            xtts = {}
            xtts[0] = xp.tile([128, kt, tc_sz], F16, tag="xtt", name="xt_0")

            def x0q(j):
                qn = kt // 4
                src0 = xt_d[0].rearrange("p (s t) -> p s t", s=kt)
                eng = nc.sync if j % 2 == 0 else nc.scalar
                eng.dma_start(xtts[0][:, j * qn:(j + 1) * qn],
                              src0[:, j * qn:(j + 1) * qn])

            bias_sb = cst.tile([128, o_c], F32, tag="bias")
            nc.gpsimd.dma_start(bias_sb[:], br_d[:])

            for ci in (2, 0, 1):
                wts[ci] = wp.tile([128, kt, maxw], F16, tag="W", name=f"W{ci}")

            # chunk2 dequant first, fine-grained (head-critical); the first
            # batches interleave with x0 quarters so the first matmul's
            # operands lead both rings. Then phase-1 x tiles prefetch.
            dq_batch(2, 0, 2)
            x0q(0); x0q(1)
            dq_batch(2, 1, 2)
            x0q(2); x0q(3)
            for b in range(2, kt // 2):
                dq_batch(2, b, 2)
            for tci in range(1, N1):
                xtts[tci] = xp.tile([128, kt, tc_sz], F16, tag="xtt",
                                    name=f"xt_{tci}")
                x_dma(xtts[tci], tci)

            # phase 1: tc0..N1-1 on chunk 2
            for tci in range(N1):
                for tl in range(tl_per_tc):
                    mm_group(xtts[tci], tci, tl, 2)
                if tci == 0:
                    for b in range(kt // 4):
                        dq_batch(0, b, 4)

            # phase 1b/1c: same resident x tiles on chunks 0 then 1 —
            # zero x traffic while the remaining dequant finishes
            for ci in (0, 1):
                for tci in range(N1):
                    for tl in range(tl_per_tc):
                        mm_group(xtts[tci], tci, tl, ci)
                if ci == 0:
                    for b in range(kt // 4):
                        dq_batch(1, b, 4)

            # phase 2: remaining token-chunks over all 3 chunks, x read
            # once; per-chunk y writes so the drain overlaps later chunks
            for tci in range(N1, n_tc):
                xtt = xp.tile([128, kt, tc_sz], F16, tag="xtt",
                              name=f"xt_{tci}")
                x_dma(xtt, tci)
                order = (0, 2, 1) if tci < N1 + 5 else (0, 1, 2)
                for tl in range(tl_per_tc):
                    for ci in order:
                        mm_group(xtt, tci, tl, ci)
    nc.compile()
    return nc


def _prep_host_inputs(x, weight_q4, weight_norm, bias):
    """Host-side shard + layout prep. Returns in_maps for 8 cores."""
    n_tc = TOKENS // TC
    xt = (x.T.astype(np.float16).reshape(KT, 128, n_tc, TC)
          .transpose(2, 1, 0, 3).reshape(n_tc, 128, KT * TC))
    xt = np.ascontiguousarray(xt)

    # nibble-unpack + transpose: wqT[k, o] = 4-bit code of W[o, k]
    b = weight_q4.reshape(OUT, BLOCKS * HALF).astype(np.uint8)
    q = np.empty((OUT, IN), np.uint8)
    q[:, 0::2] = b & 15
    q[:, 1::2] = b >> 4
    # [p, slab, o] with k = slab*128 + p
    qP = np.ascontiguousarray(q.T.reshape(KT, 128, OUT).transpose(1, 0, 2))

    # scale replication: s[k, o] = norm[o, k//16]
    sT = np.repeat(
        weight_norm.reshape(OUT, BLOCKS).T.astype(np.float16),
        GROUP, axis=0).reshape(KT, 128, OUT)
    sP = np.ascontiguousarray(sT.transpose(1, 0, 2))

    bias = bias.astype(np.float32)

    in_maps = []
    for c in range(N_CORES):
        o0 = c * O_C
        m = {"xt": xt,
             "bias_rep": np.ascontiguousarray(np.broadcast_to(
                 bias[o0:o0 + O_C][None, :], (128, O_C)))}
        for i, (off, w) in enumerate(O_CHUNKS):
            m[f"wq{i}"] = np.ascontiguousarray(qP[:, :, o0 + off:o0 + off + w])
            m[f"sc{i}"] = np.ascontiguousarray(sP[:, :, o0 + off:o0 + off + w])
        in_maps.append(m)
    return in_maps


_CACHE = {}


def _run(in_maps):
    if "nc" not in _CACHE:
        _CACHE["nc"] = build_bass()
    nc = _CACHE["nc"]
    res = run_bass_kernel_spmd(nc, in_maps, list(range(N_CORES)))
    return res


def kernel(x, weight_q4, weight_norm, bias):
    in_maps = _prep_host_inputs(
        np.asarray(x), np.asarray(weight_q4),
        np.asarray(weight_norm), np.asarray(bias))
    res = _run(in_maps)
    outs = [res.results[c]["y"] for c in range(N_CORES)]
    y = np.concatenate(outs, axis=1)
    return np.ascontiguousarray(y.astype(np.float32))


# revision 18
# speedup vs baseline: 1.1929x; 1.1929x over previous
"""Trainium2 Bass kernel for 4-bit-quantized Linear: y = x @ dequant(Wq4).T + bias.

Sharding: tensor-parallel over out_features (11008 rows -> 8 cores x 1376,
exact split), x replicated (fed pre-transposed fp16), outputs concatenated
on host.

Per-core device kernel (v4):
  - Host pre-unpacks the int4 nibbles to a transposed uint8 layout
    wqT[k, o] (pure layout transform; values stay 4-bit codes), plus a
    norm replication s[k, o] = norm[o, k//16] (fp16).
  - Device dequant is k-major, PE-free: ACT z = (2q-15)/15, DVE W = z*s,
    written straight into the fp16 weight slabs the matmul streams from.
  - Matmul phases: (1) first N1 token-chunks on o-chunk 2 only (its W is
    smallest and dequantizes first), (2) remaining token-chunks over all
    3 o-chunks with a single x read, (3) first N1 token-chunks on
    o-chunks 0,1 (x re-read for those only).
  - fp16 matmul (PSUM fp32 accumulation over K=4096) + bias add.
"""
import numpy as np

import concourse.bass as bass
import concourse.bacc as bacc
import concourse.mybir as mybir
import concourse.tile as tile
from concourse.bass_utils import run_bass_kernel_spmd

F16, F32, U8 = mybir.dt.float16, mybir.dt.float32, mybir.dt.uint8

# Problem constants (hardcoded per contract)
TOKENS, IN, OUT = 4096, 4096, 11008
GROUP, BLOCKS, HALF = 16, 256, 8
N_CORES = 8
O_C = OUT // N_CORES            # 1376 per-core out rows, exact
KT = IN // 128                  # 32 k-slabs
TC = 256                        # tokens per x-DMA super-chunk
O_CHUNKS = [(0, 512), (512, 512), (1024, 352)]   # (offset, width)
N1 = 3                          # head token-chunks processed per-phase
KB = 4                          # k-slabs per dequant batch


def build_bass(tokens=TOKENS, in_=IN, o_c=O_C, tc_sz=TC, o_chunks=None):
    kt = in_ // 128
    if o_chunks is None:
        o_chunks = O_CHUNKS
    n_tc = tokens // tc_sz
    tl_per_tc = tc_sz // 128
    maxw = max(w for _, w in o_chunks)

    nc = bacc.Bacc("TRN2", target_bir_lowering=False, debug=False)

    xt_d = nc.dram_tensor("xt", [n_tc, 128, kt * tc_sz], F16, kind="ExternalInput")
    wq_ds = [nc.dram_tensor(f"wq{i}", [128, kt, w], U8, kind="ExternalInput")
             for i, (_, w) in enumerate(o_chunks)]
    sc_ds = [nc.dram_tensor(f"sc{i}", [128, kt, w], F16, kind="ExternalInput")
             for i, (_, w) in enumerate(o_chunks)]
    br_d = nc.dram_tensor("bias_rep", [128, o_c], F32, kind="ExternalInput")
    y_d = nc.dram_tensor("y", [tokens, o_c], F32, kind="ExternalOutput")

    with tile.TileContext(nc) as tc:
        with (
            tc.tile_pool(name="const", bufs=1) as cst,
            tc.tile_pool(name="wp", bufs=3) as wp,
            tc.tile_pool(name="q8", bufs=8) as q8p,
            tc.tile_pool(name="sc", bufs=6) as scp,
            tc.tile_pool(name="xp", bufs=3) as xp,
            tc.tile_pool(name="yp", bufs=3) as yp,
            tc.tile_pool(name="psm", bufs=6, space=bass.MemorySpace.PSUM) as psm,
            tc.tile_pool(name="psd", bufs=1, space=bass.MemorySpace.PSUM) as psd,
        ):
            wts = {}
            par = [0]

            def dq_batch(ci, b, kb):
                """Dequantize kb k-slabs of chunk ci into wts[ci] in place."""
                o_off, o_w = o_chunks[ci]
                k0 = b * kb
                p = par[0]; par[0] += 1
                # spread transfers across DMA rings; head chunk (c2) stays
                # off the gpsimd ring, which carries bias/y at the start.
                if ci == 2:
                    qeng = nc.sync if p % 2 == 0 else nc.scalar
                    seng = nc.scalar if p % 2 == 0 else nc.sync
                else:
                    rot = (nc.sync, nc.scalar, nc.gpsimd)
                    qeng = rot[p % 3]
                    seng = rot[(p + 1) % 3]
                q = q8p.tile([128, 4, maxw], U8, tag="q")
                qeng.dma_start(q[:, :kb, :o_w], wq_ds[ci][:, k0:k0 + kb, :])
                s = scp.tile([128, 4, maxw], F16, tag="s")
                seng.dma_start(s[:, :kb, :o_w], sc_ds[ci][:, k0:k0 + kb, :])
                wsl = wts[ci][:, k0:k0 + kb, :o_w]
                # z = (2q - 15)/15 = q*(2/15) - 1, written straight into W.
                # Head chunk (c2) splits compute across vector+gpsimd for
                # latency; c0/c1 stay on vector so gpsimd's DMA-issue stream
                # (their third ring) never blocks behind compute.
                zeng = (nc.vector if (ci != 2 or p % 2 == 0) else nc.gpsimd)
                zeng.tensor_scalar(
                    wsl, q[:, :kb, :o_w], 2.0 / 15.0, -1.0,
                    mybir.AluOpType.mult, mybir.AluOpType.add)
                zeng.tensor_tensor(
                    wsl, wsl, s[:, :kb, :o_w], mybir.AluOpType.mult)

            def mm_group(xtt, tci, tl, ci, y_sb=None, y_off=0):
                """One psum accumulation group + bias add (+ y dma if own)."""
                o_off, o_w = o_chunks[ci]
                ps = psm.tile([128, maxw], F32, tag="ps")
                for k in range(kt):
                    nc.tensor.matmul(
                        ps[:, :o_w],
                        xtt[:, k, tl * 128:(tl + 1) * 128],
                        wts[ci][:, k, :o_w],
                        start=(k == 0), stop=(k == kt - 1))
                own = y_sb is None
                if own:
                    y_sb = yp.tile([128, maxw], F32, tag="y", name="ys")
                    y_off = o_off
                nc.vector.tensor_tensor(
                    y_sb[:, o_off - y_off:o_off - y_off + o_w],
                    ps[:, :o_w],
                    bias_sb[:, o_off:o_off + o_w], mybir.AluOpType.add)
                if own:
                    yeng = nc.sync if (tci + tl) % 2 == 0 else nc.scalar
                    yeng.dma_start(
                        y_d[tci * tc_sz + tl * 128:
                            tci * tc_sz + (tl + 1) * 128,
                            o_off:o_off + o_w],
                        y_sb[:, :o_w])

            def x_dma(xtt, tci, quarters=False):
                # split transfers across the two HWDGE rings
                src = xt_d[tci].rearrange("p (s t) -> p s t", s=kt)
                if quarters:
                    qn = kt // 4
                    for j in range(4):
                        eng = nc.sync if j % 2 == 0 else nc.scalar
                        eng.dma_start(xtt[:, j * qn:(j + 1) * qn],
                                      src[:, j * qn:(j + 1) * qn])
                else:
                    nc.sync.dma_start(xtt[:, :kt // 2], src[:, :kt // 2])
                    nc.scalar.dma_start(xtt[:, kt // 2:], src[:, kt // 2:])

            # PE warm-up: dummy matmuls fill the dequant head so the HAM
            # clock gate reaches 8/8 before the first real matmul
            dmy = cst.tile([128, 128], F16, tag="dmy", name="dmy")
            nc.gpsimd.memset(dmy[:], 0)
            dps = psd.tile([128, 128], F32, tag="dps", name="dps")
            for _ in range(80):
                nc.tensor.matmul(dmy_out := dps[:, :], dmy[:], dmy[:],
                                 start=True, stop=True)

            # first x tile + bias lead the queues
            xtts = {}
            xtts[0] = xp.tile([128, kt, tc_sz], F16, tag="xtt", name="xt_0")

            def x0q(j):
                qn = kt // 4
                src0 = xt_d[0].rearrange("p (s t) -> p s t", s=kt)
                eng = nc.sync if j % 2 == 0 else nc.scalar
                eng.dma_start(xtts[0][:, j * qn:(j + 1) * qn],
                              src0[:, j * qn:(j + 1) * qn])

            bias_sb = cst.tile([128, o_c], F32, tag="bias")
            nc.gpsimd.dma_start(bias_sb[:], br_d[:])

            for ci in (2, 0, 1):
                wts[ci] = wp.tile([128, kt, maxw], F16, tag="W", name=f"W{ci}")

            # chunk2 dequant first, fine-grained (head-critical); the first
            # batches interleave with x0 quarters so the first matmul's
            # operands lead both rings. Then phase-1 x tiles prefetch.
            dq_batch(2, 0, 2)
            x0q(0); x0q(1)
            dq_batch(2, 1, 2)
            x0q(2); x0q(3)
            for b in range(2, kt // 2):
                dq_batch(2, b, 2)
            for tci in range(1, N1):
                xtts[tci] = xp.tile([128, kt, tc_sz], F16, tag="xtt",
                                    name=f"xt_{tci}")
                x_dma(xtts[tci], tci)

            # phase 1: tc0..N1-1 on chunk 2
            for tci in range(N1):
                for tl in range(tl_per_tc):
                    mm_group(xtts[tci], tci, tl, 2)
                if tci == 0:
                    for b in range(kt // 4):
                        dq_batch(0, b, 4)

            # phase 1b/1c: same resident x tiles on chunks 0 then 1 —
            # zero x traffic while the remaining dequant finishes
            for ci in (0, 1):
                for tci in range(N1):
                    for tl in range(tl_per_tc):
                        mm_group(xtts[tci], tci, tl, ci)
                if ci == 0:
                    for b in range(kt // 4):
                        dq_batch(1, b, 4)

            # phase 2: remaining token-chunks over all 3 chunks, x read
            # once; per-chunk y writes so the drain overlaps later chunks
            for tci in range(N1, n_tc):
                xtt = xp.tile([128, kt, tc_sz], F16, tag="xtt",
                              name=f"xt_{tci}")
                x_dma(xtt, tci)
                order = (0, 2, 1) if tci < N1 + 5 else (0, 1, 2)
                for tl in range(tl_per_tc):
                    for ci in order:
                        mm_group(xtt, tci, tl, ci)
    nc.compile()
    return nc


def _prep_host_inputs(x, weight_q4, weight_norm, bias):
    """Host-side shard + layout prep. Returns in_maps for 8 cores."""
    n_tc = TOKENS // TC
    xt = (x.T.astype(np.float16).reshape(KT, 128, n_tc, TC)
          .transpose(2, 1, 0, 3).reshape(n_tc, 128, KT * TC))
    xt = np.ascontiguousarray(xt)

    # nibble-unpack + transpose: wqT[k, o] = 4-bit code of W[o, k]
    b = weight_q4.reshape(OUT, BLOCKS * HALF).astype(np.uint8)
    q = np.empty((OUT, IN), np.uint8)
    q[:, 0::2] = b & 15
    q[:, 1::2] = b >> 4
    # [p, slab, o] with k = slab*128 + p
    qP = np.ascontiguousarray(q.T.reshape(KT, 128, OUT).transpose(1, 0, 2))

    # scale replication: s[k, o] = norm[o, k//16]
    sT = np.repeat(
        weight_norm.reshape(OUT, BLOCKS).T.astype(np.float16),
        GROUP, axis=0).reshape(KT, 128, OUT)
    sP = np.ascontiguousarray(sT.transpose(1, 0, 2))

    bias = bias.astype(np.float32)

    in_maps = []
    for c in range(N_CORES):
        o0 = c * O_C
        m = {"xt": xt,
             "bias_rep": np.ascontiguousarray(np.broadcast_to(
                 bias[o0:o0 + O_C][None, :], (128, O_C)))}
        for i, (off, w) in enumerate(O_CHUNKS):
            m[f"wq{i}"] = np.ascontiguousarray(qP[:, :, o0 + off:o0 + off + w])
            m[f"sc{i}"] = np.ascontiguousarray(sP[:, :, o0 + off:o0 + off + w])
        in_maps.append(m)
    return in_maps


_CACHE = {}


def _run(in_maps):
    if "nc" not in _CACHE:
        _CACHE["nc"] = build_bass()
    nc = _CACHE["nc"]
    res = run_bass_kernel_spmd(nc, in_maps, list(range(N_CORES)))
    return res


def kernel(x, weight_q4, weight_norm, bias):
    in_maps = _prep_host_inputs(
        np.asarray(x), np.asarray(weight_q4),
        np.asarray(weight_norm), np.asarray(bias))
    res = _run(in_maps)
    outs = [res.results[c]["y"] for c in range(N_CORES)]
    y = np.concatenate(outs, axis=1)
    return np.ascontiguousarray(y.astype(np.float32))
